# revision 13
# baseline (speedup 1.0000x reference)
"""Trainium2 Bass kernel for soft-KNN OOD scoring (retrieval_knn).

ood[b] = sum_n softmax_n(-dist[b,n]/T) * dist[b,n],
dist = cdist(l2norm(latent_eval), l2norm(train_latents)).

Strategy (8 NeuronCores, shard train_latents along N):
  host:   l2-normalize q and t (the nn.Module does this once in __init__),
          cast bf16, shard t into 8 x [12544, 256] (zero-padded rows).
  device: DMA-xbar-transpose t-shard -> [256, 12544] SBUF, matmul with
          q^T -> PSUM cosine tiles, ACT Sqrt(200-200c) = dist/T ->
          ACT Exp(-dist/T) with fused row-accumulate (Sum w) ->
          DVE tensor_tensor_reduce w*(dist/T) fused accumulate (Sum w*d).
          No softmax max-subtraction needed: logits in [-20,-9] are exact
          in fp32.  Sqrt and Exp live in different ACT table sets, so the
          shard is processed in 2 groups (all sqrts, then all exps) to
          amortize table loads.
  host:   sum per-core partials, ood = T * Swd / Sw.
"""

import os
import sys

import numpy as np

for _p in ("/opt/trn_rl_repo", "/root/.axon_site/_ro/trn_rl_repo"):
    if os.path.isdir(_p) and _p not in sys.path:
        sys.path.insert(0, _p)

import ml_dtypes  # noqa: E402

import concourse.bass as bass  # noqa: E402, F401
import concourse.tile as tile  # noqa: E402
from concourse import bacc, mybir  # noqa: E402
from concourse.bass_utils import run_bass_kernel_spmd  # noqa: E402
from concourse.tile import add_dep_helper  # noqa: E402

BF16 = ml_dtypes.bfloat16

N_CORES = 8
B = 1024  # eval rows
D = 256  # feature dim
N = 100000  # memory bank rows
NS = N // N_CORES  # 12500 rows per core
NP = 12544  # padded shard rows (98 * 128)
TEMP = 0.1
PC = 2048  # psum chunk columns (4 PSUM banks)
MM_N = 512  # moving-operand free dim per matmul

# Diagnostics from the most recent device run (filled by _run_device).
LAST = {}
TRACE = False


def _pcs_for(gn):
    """Split a group's column count into psum-chunk widths."""
    out = []
    o = 0
    while o < gn:
        w = min(PC, gn - o)
        out.append((o, w))
        o += w
    return out


def build_program(np_pad=NP, b=B, d=D):
    """Build + compile the per-core SPMD program. Returns (nc, meta)."""
    assert np_pad % 2 == 0 and b % 128 == 0 and d % 128 == 0
    nb = b // 128
    nk = d // 128
    gn = np_pad // 2  # columns per table-phase group
    pcs = _pcs_for(gn)
    npcs = len(pcs)
    ncols = nb * 2 * npcs  # partial-stat columns

    nc = bacc.Bacc(
        "TRN2",
        target_bir_lowering=False,
        debug=False,
        enable_asserts=False,
        num_devices=N_CORES,
    )
    t_sh = nc.dram_tensor("tsh", [np_pad, d], mybir.dt.bfloat16, kind="ExternalInput").ap()
    q_t = nc.dram_tensor("qT", [d, b], mybir.dt.bfloat16, kind="ExternalInput").ap()
    parts = nc.dram_tensor("parts", [128, 2 * ncols], mybir.dt.float32, kind="ExternalOutput").ap()

    f32 = mybir.dt.float32
    bf16 = mybir.dt.bfloat16
    Sqrt = mybir.ActivationFunctionType.Sqrt
    Exp = mybir.ActivationFunctionType.Exp
    mult = mybir.AluOpType.mult
    add = mybir.AluOpType.add

    with tile.TileContext(nc) as tc:
        with (
            tc.tile_pool(name="const", bufs=1) as const_pool,
            tc.tile_pool(name="dbuf", bufs=1) as d_pool,
            tc.tile_pool(name="psum", bufs=2, space="PSUM") as psum_pool,
            tc.tile_pool(name="wbuf", bufs=6) as w_pool,
            tc.tile_pool(name="wdbuf", bufs=2) as wd_pool,
        ):
            # q^T resident: [128, nk, b]
            qt_sb = const_pool.tile([128, nk, b], bf16)
            nc.sync.dma_start(out=qt_sb, in_=q_t.rearrange("(k p) b -> p k b", p=128))

            # t^T resident: [128, nk, np_pad], filled by xbar DMA transpose.
            # Chunk fine and interleave k so the first matmuls' operands
            # (both k-halves of the first columns) land first.
            tt_sb = const_pool.tile([128, nk, np_pad], bf16)
            nch = 8
            tch = np_pad // nch
            assert tch % 16 == 0
            for ci in range(nch):
                for k in range(nk):
                    r0 = ci * tch
                    nc.sync.dma_start_transpose(
                        out=tt_sb[:, k, r0 : r0 + tch],
                        in_=t_sh[r0 : r0 + tch, k * 128 : (k + 1) * 128],
                    )

            # per-(group, btile, chunk) stat partials, written via accum_out
            parts_sb = const_pool.tile([128, 2 * ncols], f32)

            # bias for Sqrt(200 - 200c): per-partition scalar 200.0
            bias200 = const_pool.tile([128, 1], f32)
            nc.vector.memset(bias200, 2.0 / (TEMP * TEMP))

            # dist/T staging for one group, one tile per b-tile so the
            # next group's sqrt writes only WAR-wait on this b-tile's
            # readers (finer cross-phase overlap).
            d_tiles = [
                d_pool.tile([128, gn], bf16, name=f"dsb{bt}", tag=f"dsb{bt}")
                for bt in range(nb)
            ]

            # The tile scheduler is table-set-blind and will happily
            # interleave Sqrt and Exp ops, paying a ~2.7us ACT_TABLE_LOAD
            # per switch (measured: 64 ATLs without this).  Chain every
            # ACT op after the previous one (same-engine ordering edge,
            # no semaphore) so the sqrt->exp phase structure survives
            # scheduling and only 4 table loads remain.
            prev_act = [None]

            def chain_act(h):
                inst = getattr(h, "ins", h)
                if prev_act[0] is not None:
                    add_dep_helper(inst, prev_act[0], False, "act table phase order")
                prev_act[0] = inst
                return h

            for g in range(2):
                gbase = g * gn
                # ---- sqrt phase (matmul -> psum -> ACT Sqrt -> d_sb) ----
                for bt in range(nb):
                    for pci, (po, pw) in enumerate(pcs):
                        ps = psum_pool.tile([128, PC], f32)
                        for k in range(nk):
                            nn = 0
                            while nn < pw:
                                w = min(MM_N, pw - nn)
                                nc.tensor.matmul(
                                    ps[:, nn : nn + w],
                                    qt_sb[:, k, bt * 128 : (bt + 1) * 128],
                                    tt_sb[:, k, gbase + po + nn : gbase + po + nn + w],
                                    start=(k == 0),
                                    stop=(k == nk - 1),
                                )
                                nn += w
                        # d/T = sqrt(200 - 200 * cos)
                        chain_act(nc.scalar.activation(
                            d_tiles[bt][:, po : po + pw],
                            ps[:, :pw],
                            Sqrt,
                            bias=bias200[:, :],
                            scale=-2.0 / (TEMP * TEMP),
                        ))
                # ---- exp phase (ACT Exp + accum, DVE w*d + accum) ----
                for bt in range(nb):
                    for pci, (po, pw) in enumerate(pcs):
                        idx = (bt * 2 + g) * npcs + pci
                        wt = w_pool.tile([128, PC], bf16)
                        chain_act(nc.scalar.activation(
                            wt[:, :pw],
                            d_tiles[bt][:, po : po + pw],
                            Exp,
                            scale=-1.0,
                            accum_out=parts_sb[:, idx : idx + 1],
                        ))
                        wd = wd_pool.tile([128, PC], bf16)
                        nc.vector.scalar_tensor_tensor(
                            out=wd[:, :pw],
                            in0=wt[:, :pw],
                            scalar=1.0,
                            in1=d_tiles[bt][:, po : po + pw],
                            op0=mult,
                            op1=mult,
                            accum_out=parts_sb[:, ncols + idx : ncols + idx + 1],
                        )

            nc.sync.dma_start(out=parts, in_=parts_sb)

    nc.compile()
    meta = dict(nb=nb, npcs=npcs, ncols=ncols)
    return nc, meta


_PROG_CACHE = {}


def _get_program(np_pad=NP, b=B, d=D):
    key = (np_pad, b, d)
    if key not in _PROG_CACHE:
        _PROG_CACHE[key] = build_program(np_pad, b, d)
    return _PROG_CACHE[key]


def _run_device(shards, q_t, np_pad=NP, b=B, d=D):
    """shards: list of [np_pad, d] bf16; q_t: [d, b] bf16.
    Returns summed partial stats array [128, 2*ncols] (fp32) and meta."""
    nc, meta = _get_program(np_pad, b, d)
    in_maps = [{"tsh": sh, "qT": q_t} for sh in shards]
    res = run_bass_kernel_spmd(
        nc, in_maps, core_ids=list(range(len(shards))), trace=TRACE
    )
    LAST["exec_time_ns"] = res.exec_time_ns
    LAST["profile_json"] = res.profile_json
    total = np.zeros((128, 2 * meta["ncols"]), np.float32)
    for core_out in res.results:
        total += np.asarray(core_out["parts"], np.float32)
    return total, meta


def kernel(latent_eval, train_latents):
    q = np.asarray(latent_eval, dtype=np.float32)
    t = np.asarray(train_latents, dtype=np.float32)
    assert q.shape == (B, D) and t.shape == (N, D)

    # Module-__init__-style normalization on host, bf16 for the PE array.
    qn = q / np.maximum(np.linalg.norm(q, axis=1, keepdims=True), 1e-12)
    tn = t / np.maximum(np.linalg.norm(t, axis=1, keepdims=True), 1e-12)
    q_t = np.ascontiguousarray(qn.T).astype(BF16)  # [D, B]
    tnb = tn.astype(BF16)

    shards = []
    for c in range(N_CORES):
        sh = np.zeros((NP, D), BF16)
        sh[:NS] = tnb[c * NS : (c + 1) * NS]
        shards.append(sh)

    total, meta = _run_device(shards, q_t)

    nb, npcs, ncols = meta["nb"], meta["npcs"], meta["ncols"]
    # column idx = (bt*2 + g)*npcs + pci ; row p -> b = bt*128 + p
    sw = total[:, :ncols].reshape(128, nb, 2 * npcs).sum(axis=2)  # [128, nb]
    swd = total[:, ncols:].reshape(128, nb, 2 * npcs).sum(axis=2)
    sw_b = sw.T.reshape(-1)  # b = bt*128 + p
    swd_b = swd.T.reshape(-1)
    ood = TEMP * swd_b / sw_b
    return ood.astype(np.float32)
